# revision 54
# baseline (speedup 1.0000x reference)
"""Trainium2 Bass kernel for 16-head MultiHeadAttention (B=2, S=2048, D=1024).

Sharding: 8 cores = 2 (batch) x 4 (head groups of 4 heads). TP over heads;
the host sums the 4 out-projection partials per batch element.

v2 design (vs the v1 baseline at 274915 ns):
- x and all weights are pre-cast to bf16 and pre-transposed into the exact
  SBUF layouts on the host: no on-device casts, no transpose DMAs, and the
  PE can start within a few microseconds.
- PV uses exp(scores) as the matmul stationary ([128 keys, 128 queries])
  with V' as the 65-column moving operand, so every PE column streams a
  full 128-row contraction (the v1 layout wasted half the array).
- K bias is dropped entirely (it is softmax-invariant); the denominator
  rides in V' as a 65th ones-column, and normalization is a per-partition
  reciprocal+scale on the DVE.
- K/V projection chunks are interleaved with the first attention pass so
  the activation engine (exp is ~48% of the runtime floor) starts early
  and never starves.
- Output is stored as bf16 and reduced in fp32 on the host.

Self-contained: hardcodes shapes; only dependency is the in-container
concourse/bass stack at /opt/trn_rl_repo.
"""

import sys

for _p in ("/opt/trn_rl_repo",):
    if _p not in sys.path:
        sys.path.insert(0, _p)

import ml_dtypes
import numpy as np

import concourse.bass as bass  # noqa: E402,F401
import concourse.bacc as bacc  # noqa: E402
import concourse.tile as tile  # noqa: E402
from concourse import mybir  # noqa: E402
from concourse.bass_utils import run_bass_kernel_spmd  # noqa: E402

F32 = mybir.dt.float32
BF16 = mybir.dt.bfloat16
AF = mybir.ActivationFunctionType
BF = ml_dtypes.bfloat16

S = 2048        # sequence length
DIN = 1024      # model dim
HPC = 4         # heads per core
DK = 64         # head dim
N_CORES = 8
SC = 4          # s-chunks for projection
CS = 512        # s-chunk width
KC = 8          # DIN contraction chunks of 128
SB = 16         # 128-row s-blocks
SQH = 4         # attention query passes
SQC = 512      # queries per pass
QB = 4          # 128-query blocks per pass
SCALE_INV = 1.0 / 8.0  # 1/sqrt(DK)


def build_nc(taps=False):
    nc = bacc.Bacc("TRN2", target_bir_lowering=False, debug=False,
                   num_devices=N_CORES)

    xt_d = nc.dram_tensor("xt", [128, SC * KC * CS], BF16, kind="ExternalInput")
    wqk_d = nc.dram_tensor("wqk", [128, 4 * KC * 128], BF16, kind="ExternalInput")
    wv_d = nc.dram_tensor("wv", [128, KC * 256], BF16, kind="ExternalInput")
    wo_d = nc.dram_tensor("wo", [128, 2 * DIN], BF16, kind="ExternalInput")
    bq_d = nc.dram_tensor("bq", [128, 2], F32, kind="ExternalInput")
    bv_d = nc.dram_tensor("bv", [128, 2 * 256], BF16, kind="ExternalInput")
    bo_d = nc.dram_tensor("bo", [128, DIN], BF16, kind="ExternalInput")
    id_d = nc.dram_tensor("ident", [128, 128], BF16, kind="ExternalInput")
    out_d = nc.dram_tensor("out", [S, DIN], BF16, kind="ExternalOutput")
    if taps:
        tap_qk_d = nc.dram_tensor("tap_qk", [128, 4 * S], BF16,
                                  kind="ExternalOutput")
        tap_vv_d = nc.dram_tensor("tap_vv", [128, SB * HPC * 65], BF16,
                                  kind="ExternalOutput")
        tap_atn_d = nc.dram_tensor("tap_atn", [128, SB * 256], BF16,
                                   kind="ExternalOutput")
        tap_atT_d = nc.dram_tensor("tap_atT", [128, 2 * S], BF16,
                                   kind="ExternalOutput")

    with tile.TileContext(nc) as tc:
        with (
            tc.tile_pool(name="persist", bufs=1) as pers,
            tc.tile_pool(name="exps", bufs=48) as exp_pool,
            tc.tile_pool(name="outs", bufs=4) as ot_pool,
            tc.tile_pool(name="rcs", bufs=4) as rc_pool,
            tc.tile_pool(name="ps", bufs=2, space="PSUM") as ps_pool,
            tc.tile_pool(name="pp", bufs=1, space="PSUM") as pp_pool,
            tc.tile_pool(name="po", bufs=1, space="PSUM") as po_pool,
            tc.tile_pool(name="tp", bufs=1, space="PSUM") as tp_pool,
        ):
            # ---- persistent SBUF tensors ----
            xt = pers.tile([128, SC * KC * CS], BF16, tag="xt")
            wqk = pers.tile([128, 4 * KC * 128], BF16, tag="wqk")
            wv = pers.tile([128, KC * 256], BF16, tag="wv")
            wo = pers.tile([128, 2 * DIN], BF16, tag="wo")
            qk = pers.tile([128, 4 * S], BF16, tag="qk")   # k01,k23,q01,q23
            vv = pers.tile([128, SB * HPC * 65], BF16, tag="vv")
            atn = pers.tile([128, SB * 256], BF16, tag="atn")  # [q, (qbg, hd)]
            atT = pers.tile([128, 2 * S], BF16, tag="atT")     # [(ch), s]
            bq = pers.tile([128, 2], F32, tag="bq")
            bv4 = pers.tile([128, 2 * 256], BF16, tag="bv4")
            bo128 = pers.tile([128, DIN], BF16, tag="bo128")
            ident = pers.tile([128, 128], BF16, tag="ident")

            xtv = xt[:].rearrange("p (sc c j) -> p sc c j", sc=SC, c=KC)
            wqkv = wqk[:].rearrange("p (b c m) -> p b c m", b=4, c=KC)
            wvv = wv[:].rearrange("p (c d) -> p c d", c=KC)
            wov = wo[:].rearrange("p (ch d) -> p ch d", ch=2)
            qkv4 = qk[:].rearrange("p (b s) -> p b s", b=4)

            # ---- loads, ordered for earliest PE start ----
            nc.sync.dma_start(wqk[:, 0:1024], wqk_d.ap()[:, 0:1024])   # k01
            nc.sync.dma_start(xt[:, 0:2048], xt_d.ap()[:, 0:2048])     # sc0 c0-3
            nc.sync.dma_start(xt[:, 2048:4096], xt_d.ap()[:, 2048:4096])
            nc.sync.dma_start(wqk[:, 2048:3072], wqk_d.ap()[:, 2048:3072])  # q01
            nc.sync.dma_start(bq[:], bq_d.ap())
            nc.sync.dma_start(xt[:, KC * CS:2 * KC * CS],
                              xt_d.ap()[:, KC * CS:2 * KC * CS])       # sc1
            nc.sync.dma_start(xt[:, 2 * KC * CS:3 * KC * CS],
                              xt_d.ap()[:, 2 * KC * CS:3 * KC * CS])   # sc2
            nc.sync.dma_start(wqk[:, 1024:2048], wqk_d.ap()[:, 1024:2048])  # k23
            nc.sync.dma_start(wqk[:, 3072:4096], wqk_d.ap()[:, 3072:4096])  # q23
            nc.sync.dma_start(wv[:], wv_d.ap())
            nc.sync.dma_start(xt[:, 3 * KC * CS:4 * KC * CS],
                              xt_d.ap()[:, 3 * KC * CS:4 * KC * CS])   # sc3
            nc.sync.dma_start(bv4[:], bv_d.ap())
            nc.sync.dma_start(ident[:], id_d.ap())
            nc.sync.dma_start(wo[:], wo_d.ap())
            nc.sync.dma_start(bo128[:], bo_d.ap())

            # ones columns of V' (written once; V values land around them)
            vvv = vv[:].rearrange("p (n e) -> p n e", e=65)
            nc.vector.memset(vvv[:, :, 64:65], 1.0)

            def proj_qk(sc, blk, pool_tag=None):
                """One Q^T/K^T block (k01,k23,q01,q23) for s-chunk sc."""
                if pool_tag == "ps":
                    t = ps_pool.tile([128, 1024], F32, tag="ps")
                else:
                    t = pp_pool.tile([128, 512], F32, tag="pp")
                for c in range(KC):
                    nc.tensor.matmul(
                        t[:, 0:512],
                        wqkv[:, blk, c, :],
                        xtv[:, sc, c, :],
                        start=(c == 0), stop=(c == KC - 1))
                dst = qkv4[:, blk, sc * CS:(sc + 1) * CS]
                if blk >= 2:
                    nc.vector.tensor_scalar_add(dst, t[:, 0:512],
                                                bq[:, blk - 2:blk - 1])
                else:
                    nc.vector.tensor_copy(dst, t[:, 0:512])

            def proj_v(sc, half):
                """V natural [s, hd] + bias for 2 s-blocks of chunk sc."""
                t = pp_pool.tile([128, 512], F32, tag="pp")
                for k in range(2):
                    for c in range(KC):
                        nc.tensor.matmul(
                            t[:, k * 256:(k + 1) * 256],
                            xtv[:, sc, c, (2 * half + k) * 128:(2 * half + k + 1) * 128],
                            wvv[:, c, :],
                            start=(c == 0), stop=(c == KC - 1))
                sb0 = sc * 4 + 2 * half
                dst = vv[:].rearrange("p (n h e) -> p n h e", h=HPC, e=65)[
                    :, sb0:sb0 + 2, :, 0:64]
                src = t[:].rearrange("p (k h d) -> p k h d", k=2, h=HPC)
                b3 = bv4[:].rearrange("p (k h d) -> p k h d", k=2, h=HPC)
                nc.vector.tensor_add(dst, src, b3)

            def outproj(qbg, eng, tail=False):
                for dc in range(2):
                    # at the tail the scores slots are free: rotate through
                    # pp/psA/psB so the evacuation never stalls the PE
                    if tail and (qbg * 2 + dc) % 3 != 0:
                        t = ps_pool.tile([128, 1024], F32, tag="ps")
                    else:
                        t = pp_pool.tile([128, 512], F32, tag="pp")
                    for ch in range(2):
                        nc.tensor.matmul(
                            t[:, 0:512],
                            atT[:, ch * S + qbg * 128:ch * S + (qbg + 1) * 128],
                            wov[:, ch, dc * 512:(dc + 1) * 512],
                            start=(ch == 0), stop=(ch == 1))
                    ot = ot_pool.tile([128, 512], BF16, tag="ot")
                    eng.tensor_add(ot[:], t[:, 0:512],
                                   bo128[:, dc * 512:(dc + 1) * 512])
                    nc.sync.dma_start(
                        out_d.ap()[qbg * 128:(qbg + 1) * 128,
                                   dc * 512:(dc + 1) * 512], ot[:])

            def attn_i_step(sqh, pr, i):
                """Scores + exp for one (pair, key-block); returns the exp tile."""
                ps = ps_pool.tile([128, 1024], F32, tag="ps")
                for hl in range(2):
                    p0 = hl * 64
                    nc.tensor.matmul(
                        ps[:, hl * 512:(hl + 1) * 512],
                        qk[p0:p0 + 64, pr * S + i * 128:pr * S + (i + 1) * 128],
                        qk[p0:p0 + 64,
                           (2 + pr) * S + sqh * SQC:(2 + pr) * S + (sqh + 1) * SQC],
                        start=True, stop=True)
                ex = exp_pool.tile([128, 1024], BF16, tag="ex", name=f"ex{i}")
                nc.scalar.activation(ex[:], ps[:], AF.Exp, scale=SCALE_INV)
                return ex

            def tpose_qb(qbg):
                """Transpose one q-block's [q, hd] pair; single strided evac."""
                tp = tp_pool.tile([128, 256], BF16, tag="tp")
                for ch in range(2):
                    nc.tensor.transpose(
                        tp[:, ch * 128:(ch + 1) * 128],
                        atn[:, qbg * 256 + ch * 128:qbg * 256 + (ch + 1) * 128],
                        ident[:])
                src = tp[:].rearrange("p (ch e) -> p ch e", ch=2)
                dst = atT[:].rearrange("p (ch g e) -> p ch g e", ch=2, e=128)[
                    :, :, qbg, :]
                nc.vector.tensor_copy(dst, src)

            def drain_group_thunks(sqh, pr, po, exs, tail=False):
                """Drain thunks: 8 PV accumulation groups (start->stop
                contiguous per bank; consecutive q-blocks alternate banks so
                the normalize read never stalls the next group), each q-block
                normalized as its pair completes; pr==1 appends the
                transposes and out-projections as separate pipeline phases."""
                thunks = []
                for qb in range(QB):
                    for hl in range(2):
                        def grp(qb=qb, hl=hl):
                            h = 2 * pr + hl
                            for i in range(SB):
                                nc.tensor.matmul(
                                    po[qb % 2][:, (qb // 2) * 130 + hl * 65:
                                               (qb // 2) * 130 + (hl + 1) * 65],
                                    exs[i][:, hl * SQC + qb * 128:
                                           hl * SQC + (qb + 1) * 128],
                                    vv[:, i * 260 + h * 65:
                                       i * 260 + (h + 1) * 65],
                                    start=(i == 0), stop=(i == SB - 1))
                            if hl == 1:
                                attn_norm_qb(sqh, pr, po, qb)
                                if pr == 1 and not tail:
                                    tpose_qb(sqh * QB + qb)
                                    outproj(sqh * QB + qb, nc.vector)
                        thunks.append(grp)
                if pr == 1 and tail:
                    # norms complete while later PV groups run; the
                    # transpose/out-proj chains then pipeline with the ps
                    # slots free for rotation
                    def chains():
                        for qb in range(QB):
                            tpose_qb(sqh * QB + qb)
                        for qb in range(QB):
                            outproj(sqh * QB + qb, nc.vector, tail=True)
                    thunks.append(chains)
                return thunks

            def attn_norm_qb(sqh, pr, po, qb):
                qbg = sqh * QB + qb
                pov = po[qb % 2][:, (qb // 2) * 130:(qb // 2 + 1) * 130] \
                    .rearrange("p (hl e) -> p hl e", hl=2)
                rc = rc_pool.tile([128, 2], F32, tag="rc")
                rcv = rc[:].rearrange("p (a b) -> p a b", a=2)
                nc.vector.reciprocal(rcv, pov[:, :, 64:65])
                for hl in range(2):
                    h = 2 * pr + hl
                    nc.vector.tensor_scalar_mul(
                        atn[:, qbg * 256 + h * 64:qbg * 256 + (h + 1) * 64],
                        pov[:, hl, 0:64], rc[:, hl:hl + 1])

            # ---- emission: software-pipelined projection+attention ----
            # PE executes its stream in order, so everything is emitted as
            # micro-hooks inside the scores/exp streams: projection chunks
            # (deadline-ordered) in passes 0-2, PV-drain groups of pass n-2
            # in passes 2-6, drains of passes 5 AND 6 sharing pass 7's
            # stream (sequentially, to keep PSUM accumulation groups of one
            # bank non-interleaved), and only pass 7's drain trailing.
            proj_hooks = {
                (0, 0): [(proj_qk, 1, 0)],     # k01 c1  (scores(0,4))
                (0, 3): [(proj_qk, 2, 0)],     # k01 c2  (scores(0,8); xt2 DMA)
                (0, 5): [(proj_qk, 0, 1)],     # k23 c0  (scores(1,0))
                (0, 7): [(proj_qk, 0, 3)],     # q23 c0  (scores(1,0))
                (0, 9): [(proj_qk, 3, 0)],     # k01 c3  (scores(0,12); xt3 DMA)
                (0, 11): [(proj_qk, 1, 1)],    # k23 c1  (scores(1,4))
                (0, 13): [(proj_qk, 2, 1)],    # k23 c2  (scores(1,8))
                (0, 15): [(proj_qk, 3, 1)],    # k23 c3  (scores(1,12))
                (1, 0): [(proj_v, 0, 0)],      # V chunks (PV(0), in pass 2)
                (1, 2): [(proj_v, 0, 1)],
                (1, 4): [(proj_v, 1, 0)],
                (1, 6): [(proj_v, 1, 1)],
                (1, 8): [(proj_v, 2, 0)],
                (1, 10): [(proj_v, 2, 1)],
                (1, 13): [(proj_qk, 1, 2)],    # q01 c1  (scores(2,0))
                (2, 0): [(proj_v, 3, 0)],
                (2, 2): [(proj_v, 3, 1)],
                (2, 13): [(proj_qk, 1, 3)],    # q23 c1  (scores(3,0))
                (3, 13): [(proj_qk, 2, 2)],    # q01 c2  (scores(4,0))
                (4, 13): [(proj_qk, 2, 3)],    # q23 c2  (scores(5,0))
                (5, 13): [(proj_qk, 3, 2)],    # q01 c3  (scores(6,0))
                (6, 13): [(proj_qk, 3, 3)],    # q23 c3  (scores(7,0))
            }

            def claim_po():
                return [po_pool.tile([128, 260], F32, tag=f"po{j}",
                                     name=f"po{j}")
                        for j in range(2)]

            # PE p-state warmup: ~40 cheap transposes bridge the DMA load
            # latency so the real matmuls start at full clock.
            warm = pers.tile([128, 128], BF16, tag="warm")
            nc.vector.memset(warm[:], 1.0)
            for _ in range(20):
                tpw = tp_pool.tile([128, 256], BF16, tag="tp")
                nc.tensor.transpose(tpw[:, 0:128], warm[:], warm[:])

            proj_qk(0, 0, pool_tag="ps")    # k01 chunk 0 (idle ps slot)
            proj_qk(0, 2, pool_tag="ps")    # q01 chunk 0

            passes = []                     # exs per pass
            meta = []                       # (sqh, pr) per pass
            for idx in range(2 * SQH):
                sqh, pr = divmod(idx, 2)
                extra = {}
                for (pidx, i), fns in proj_hooks.items():
                    if pidx == idx:
                        for fn, a, b in fns:
                            extra.setdefault(i, []).append(
                                lambda fn=fn, a=a, b=b: fn(a, b))
                if 2 <= idx <= 6:
                    j = idx - 2
                    po_j = claim_po()
                    # pass 2 carries the V chunk-3 hooks at i=0/2; its drain
                    # groups must come after them (they read all of V')
                    off = 4 if idx == 2 else 1
                    step = 1 if idx == 2 else 2
                    for g, th in enumerate(
                            drain_group_thunks(*meta[j], po_j, passes[j])):
                        extra.setdefault(min(off + step * g, SB - 1), []).append(th)
                if idx == 7:
                    po_5 = claim_po()
                    for g, th in enumerate(
                            drain_group_thunks(*meta[5], po_5, passes[5])):
                        extra.setdefault(g // 2, []).append(th)
                    po_6 = claim_po()
                    for g, th in enumerate(
                            drain_group_thunks(*meta[6], po_6, passes[6])):
                        extra.setdefault(min(6 + g, SB - 1), []).append(th)
                exs = []
                for i in range(SB):
                    exs.append(attn_i_step(sqh, pr, i))
                    for th in extra.get(i, ()):
                        th()
                passes.append(exs)
                meta.append((sqh, pr))
            po_7 = claim_po()
            for th in drain_group_thunks(*meta[7], po_7, passes[7], tail=True):
                th()

            if taps:
                nc.sync.dma_start(tap_qk_d.ap(), qk[:])
                nc.sync.dma_start(tap_vv_d.ap(), vv[:])
                nc.sync.dma_start(tap_atn_d.ap(), atn[:])
                nc.sync.dma_start(tap_atT_d.ap(), atT[:])

    nc.compile()
    return nc


def shard_inputs(x, w_qkv, b_qkv, w_out, b_out):
    """Host-side prep: slice per core, cast to bf16, pre-transpose layouts."""
    x = np.asarray(x, dtype=np.float32)
    w_qkv = np.asarray(w_qkv, dtype=np.float32)
    b_qkv = np.asarray(b_qkv, dtype=np.float32)
    w_out = np.asarray(w_out, dtype=np.float32)
    b_out = np.asarray(b_out, dtype=np.float32)
    eye = np.eye(128, dtype=BF)
    bo_full = np.ascontiguousarray(
        np.broadcast_to(b_out.astype(BF).reshape(1, DIN), (128, DIN)))
    bo_zero = np.zeros((128, DIN), dtype=BF)

    def qkblk(W):  # [256, 1024] -> [p, half, c, m]
        return W.astype(BF).reshape(2, 128, KC, 128).transpose(3, 0, 2, 1)

    in_maps = []
    for core in range(N_CORES):
        b, hg = divmod(core, 4)
        sl = slice(hg * 256, (hg + 1) * 256)
        Wq = w_qkv[0 * DIN:1 * DIN][sl]
        Wk = w_qkv[1 * DIN:2 * DIN][sl]
        Wv = w_qkv[2 * DIN:3 * DIN][sl]
        bq_s = b_qkv[0 * DIN:1 * DIN][sl]
        bv_s = b_qkv[2 * DIN:3 * DIN][sl]
        Wo = w_out[:, sl]
        xt = np.ascontiguousarray(
            x[b].astype(BF).reshape(SC, CS, KC, 128)
            .transpose(3, 0, 2, 1).reshape(128, SC * KC * CS))
        wqk = np.ascontiguousarray(
            np.concatenate([qkblk(Wk), qkblk(Wq)], axis=1)
            .reshape(128, 4 * KC * 128))
        wv_h = np.ascontiguousarray(
            Wv.astype(BF).reshape(256, KC, 128).transpose(2, 1, 0)
            .reshape(128, KC * 256))
        wo_h = np.ascontiguousarray(
            Wo.astype(BF).reshape(DIN, 2, 128).transpose(2, 1, 0)
            .reshape(128, 2 * DIN))
        bq_h = np.ascontiguousarray(bq_s.reshape(2, 128).T, dtype=np.float32)
        bv_h = np.ascontiguousarray(
            np.tile(bv_s.astype(BF).reshape(1, 256), (128, 2)))
        in_maps.append({
            "xt": xt, "wqk": wqk, "wv": wv_h, "wo": wo_h,
            "bq": bq_h, "bv": bv_h,
            "bo": bo_full if hg == 0 else bo_zero,
            "ident": eye,
        })
    return in_maps


def gather_output(results):
    outs = []
    for b in range(2):
        acc = np.zeros((S, DIN), dtype=np.float32)
        for core in range(4 * b, 4 * b + 4):
            acc += results[core]["out"].astype(np.float32)
        outs.append(acc)
    return np.stack(outs, axis=0)


_NC_CACHE = {}


def _get_nc():
    if "nc" not in _NC_CACHE:
        _NC_CACHE["nc"] = build_nc()
    return _NC_CACHE["nc"]


def kernel(x, w_qkv, b_qkv, w_out, b_out):
    nc = _get_nc()
    in_maps = shard_inputs(x, w_qkv, b_qkv, w_out, b_out)
    res = run_bass_kernel_spmd(nc, in_maps, core_ids=list(range(N_CORES)))
    return gather_output(res.results)


if __name__ == "__main__":
    rng = np.random.default_rng(0)
    x = rng.standard_normal((2, S, DIN), dtype=np.float32)
    w_qkv = rng.standard_normal((3 * DIN, DIN), dtype=np.float32) / 32.0
    b_qkv = rng.standard_normal(3 * DIN, dtype=np.float32) * 0.02
    w_out = rng.standard_normal((DIN, DIN), dtype=np.float32) / 32.0
    b_out = rng.standard_normal(DIN, dtype=np.float32) * 0.02
    out = kernel(x=x, w_qkv=w_qkv, b_qkv=b_qkv, w_out=w_out, b_out=b_out)
    print("out", out.shape, out.dtype, float(np.abs(out).mean()))


# revision 61
# speedup vs baseline: 1.0023x; 1.0023x over previous
"""Trainium2 Bass kernel for 16-head MultiHeadAttention (B=2, S=2048, D=1024).

Sharding: 8 cores = 2 (batch) x 4 (head groups of 4 heads). TP over heads;
the host sums the 4 out-projection partials per batch element.

v2 design (vs the v1 baseline at 274915 ns):
- x and all weights are pre-cast to bf16 and pre-transposed into the exact
  SBUF layouts on the host: no on-device casts, no transpose DMAs, and the
  PE can start within a few microseconds.
- PV uses exp(scores) as the matmul stationary ([128 keys, 128 queries])
  with V' as the 65-column moving operand, so every PE column streams a
  full 128-row contraction (the v1 layout wasted half the array).
- K bias is dropped entirely (it is softmax-invariant); the denominator
  rides in V' as a 65th ones-column, and normalization is a per-partition
  reciprocal+scale on the DVE.
- K/V projection chunks are interleaved with the first attention pass so
  the activation engine (exp is ~48% of the runtime floor) starts early
  and never starves.
- Output is stored as bf16 and reduced in fp32 on the host.

Self-contained: hardcodes shapes; only dependency is the in-container
concourse/bass stack at /opt/trn_rl_repo.
"""

import sys

for _p in ("/opt/trn_rl_repo",):
    if _p not in sys.path:
        sys.path.insert(0, _p)

import ml_dtypes
import numpy as np

import concourse.bass as bass  # noqa: E402,F401
import concourse.bacc as bacc  # noqa: E402
import concourse.tile as tile  # noqa: E402
from concourse import mybir  # noqa: E402
from concourse.bass_utils import run_bass_kernel_spmd  # noqa: E402

F32 = mybir.dt.float32
BF16 = mybir.dt.bfloat16
AF = mybir.ActivationFunctionType
BF = ml_dtypes.bfloat16

S = 2048        # sequence length
DIN = 1024      # model dim
HPC = 4         # heads per core
DK = 64         # head dim
N_CORES = 8
SC = 4          # s-chunks for projection
CS = 512        # s-chunk width
KC = 8          # DIN contraction chunks of 128
SB = 16         # 128-row s-blocks
SQH = 4         # attention query passes
SQC = 512      # queries per pass
QB = 4          # 128-query blocks per pass
SCALE_INV = 1.0 / 8.0  # 1/sqrt(DK)


def build_nc(taps=False):
    nc = bacc.Bacc("TRN2", target_bir_lowering=False, debug=False,
                   num_devices=N_CORES)

    xt_d = nc.dram_tensor("xt", [128, SC * KC * CS], BF16, kind="ExternalInput")
    wqk_d = nc.dram_tensor("wqk", [128, 4 * KC * 128], BF16, kind="ExternalInput")
    wv_d = nc.dram_tensor("wv", [128, KC * 256], BF16, kind="ExternalInput")
    wo_d = nc.dram_tensor("wo", [128, 2 * DIN], BF16, kind="ExternalInput")
    bq_d = nc.dram_tensor("bq", [128, 2], F32, kind="ExternalInput")
    bv_d = nc.dram_tensor("bv", [128, 2 * 256], BF16, kind="ExternalInput")
    id_d = nc.dram_tensor("ident", [128, 128], BF16, kind="ExternalInput")
    out_d = nc.dram_tensor("out", [S, DIN], BF16, kind="ExternalOutput")
    if taps:
        tap_qk_d = nc.dram_tensor("tap_qk", [128, 4 * S], BF16,
                                  kind="ExternalOutput")
        tap_vv_d = nc.dram_tensor("tap_vv", [128, SB * HPC * 65], BF16,
                                  kind="ExternalOutput")
        tap_atn_d = nc.dram_tensor("tap_atn", [128, SB * 256], BF16,
                                   kind="ExternalOutput")
        tap_atT_d = nc.dram_tensor("tap_atT", [128, 2 * S], BF16,
                                   kind="ExternalOutput")

    with tile.TileContext(nc) as tc:
        with (
            tc.tile_pool(name="persist", bufs=1) as pers,
            tc.tile_pool(name="exps", bufs=48) as exp_pool,
            tc.tile_pool(name="outs", bufs=4) as ot_pool,
            tc.tile_pool(name="rcs", bufs=4) as rc_pool,
            tc.tile_pool(name="ps", bufs=2, space="PSUM") as ps_pool,
            tc.tile_pool(name="pp", bufs=1, space="PSUM") as pp_pool,
            tc.tile_pool(name="po", bufs=1, space="PSUM") as po_pool,
            tc.tile_pool(name="tp", bufs=1, space="PSUM") as tp_pool,
        ):
            # ---- persistent SBUF tensors ----
            xt = pers.tile([128, SC * KC * CS], BF16, tag="xt")
            wqk = pers.tile([128, 4 * KC * 128], BF16, tag="wqk")
            wv = pers.tile([128, KC * 256], BF16, tag="wv")
            wo = pers.tile([128, 2 * DIN], BF16, tag="wo")
            qk = pers.tile([128, 4 * S], BF16, tag="qk")   # k01,k23,q01,q23
            vv = pers.tile([128, SB * HPC * 65], BF16, tag="vv")
            atn = pers.tile([128, SB * 256], BF16, tag="atn")  # [q, (qbg, hd)]
            atT = pers.tile([128, 2 * S], BF16, tag="atT")     # [(ch), s]
            bq = pers.tile([128, 2], F32, tag="bq")
            bv4 = pers.tile([128, 2 * 256], BF16, tag="bv4")
            ident = pers.tile([128, 128], BF16, tag="ident")

            xtv = xt[:].rearrange("p (sc c j) -> p sc c j", sc=SC, c=KC)
            wqkv = wqk[:].rearrange("p (b c m) -> p b c m", b=4, c=KC)
            wvv = wv[:].rearrange("p (c d) -> p c d", c=KC)
            wov = wo[:].rearrange("p (ch d) -> p ch d", ch=2)
            qkv4 = qk[:].rearrange("p (b s) -> p b s", b=4)

            # ---- loads, ordered for earliest PE start ----
            nc.sync.dma_start(wqk[:, 0:1024], wqk_d.ap()[:, 0:1024])   # k01
            nc.sync.dma_start(xt[:, 0:2048], xt_d.ap()[:, 0:2048])     # sc0 c0-3
            nc.sync.dma_start(xt[:, 2048:4096], xt_d.ap()[:, 2048:4096])
            nc.sync.dma_start(wqk[:, 2048:3072], wqk_d.ap()[:, 2048:3072])  # q01
            nc.sync.dma_start(bq[:], bq_d.ap())
            nc.sync.dma_start(xt[:, KC * CS:2 * KC * CS],
                              xt_d.ap()[:, KC * CS:2 * KC * CS])       # sc1
            nc.sync.dma_start(xt[:, 2 * KC * CS:3 * KC * CS],
                              xt_d.ap()[:, 2 * KC * CS:3 * KC * CS])   # sc2
            nc.sync.dma_start(wqk[:, 1024:2048], wqk_d.ap()[:, 1024:2048])  # k23
            nc.sync.dma_start(wqk[:, 3072:4096], wqk_d.ap()[:, 3072:4096])  # q23
            nc.sync.dma_start(wv[:], wv_d.ap())
            nc.sync.dma_start(xt[:, 3 * KC * CS:4 * KC * CS],
                              xt_d.ap()[:, 3 * KC * CS:4 * KC * CS])   # sc3
            nc.sync.dma_start(bv4[:], bv_d.ap())
            nc.sync.dma_start(ident[:], id_d.ap())
            nc.sync.dma_start(wo[:], wo_d.ap())

            # ones columns of V' (written once; V values land around them)
            vvv = vv[:].rearrange("p (n e) -> p n e", e=65)
            nc.vector.memset(vvv[:, :, 64:65], 1.0)

            def proj_qk(sc, blk, pool_tag=None):
                """One Q^T/K^T block (k01,k23,q01,q23) for s-chunk sc."""
                if pool_tag == "ps":
                    t = ps_pool.tile([128, 1024], F32, tag="ps")
                else:
                    t = pp_pool.tile([128, 512], F32, tag="pp")
                for c in range(KC):
                    nc.tensor.matmul(
                        t[:, 0:512],
                        wqkv[:, blk, c, :],
                        xtv[:, sc, c, :],
                        start=(c == 0), stop=(c == KC - 1))
                dst = qkv4[:, blk, sc * CS:(sc + 1) * CS]
                if blk >= 2:
                    nc.vector.tensor_scalar_add(dst, t[:, 0:512],
                                                bq[:, blk - 2:blk - 1])
                else:
                    nc.vector.tensor_copy(dst, t[:, 0:512])

            def proj_v(sc, half):
                """V natural [s, hd] + bias for 2 s-blocks of chunk sc."""
                t = pp_pool.tile([128, 512], F32, tag="pp")
                for k in range(2):
                    for c in range(KC):
                        nc.tensor.matmul(
                            t[:, k * 256:(k + 1) * 256],
                            xtv[:, sc, c, (2 * half + k) * 128:(2 * half + k + 1) * 128],
                            wvv[:, c, :],
                            start=(c == 0), stop=(c == KC - 1))
                sb0 = sc * 4 + 2 * half
                dst = vv[:].rearrange("p (n h e) -> p n h e", h=HPC, e=65)[
                    :, sb0:sb0 + 2, :, 0:64]
                src = t[:].rearrange("p (k h d) -> p k h d", k=2, h=HPC)
                b3 = bv4[:].rearrange("p (k h d) -> p k h d", k=2, h=HPC)
                nc.vector.tensor_add(dst, src, b3)

            def outproj(qbg, eng, tail=False):
                for dc in range(2):
                    # at the tail the scores slots are free: rotate through
                    # pp/psA/psB so the evacuation never stalls the PE
                    if tail and (qbg * 2 + dc) % 3 != 0:
                        t = ps_pool.tile([128, 1024], F32, tag="ps")
                    else:
                        t = pp_pool.tile([128, 512], F32, tag="pp")
                    for ch in range(2):
                        nc.tensor.matmul(
                            t[:, 0:512],
                            atT[:, ch * S + qbg * 128:ch * S + (qbg + 1) * 128],
                            wov[:, ch, dc * 512:(dc + 1) * 512],
                            start=(ch == 0), stop=(ch == 1))
                    ot = ot_pool.tile([128, 512], BF16, tag="ot")
                    if tail and dc % 2 == 1:
                        nc.scalar.copy(ot[:], t[:, 0:512])  # ACT idle at tail
                    else:
                        eng.tensor_copy(ot[:], t[:, 0:512])
                    nc.sync.dma_start(
                        out_d.ap()[qbg * 128:(qbg + 1) * 128,
                                   dc * 512:(dc + 1) * 512], ot[:])

            def attn_i_step(sqh, pr, i):
                """Scores + exp for one (pair, key-block); returns the exp tile."""
                ps = ps_pool.tile([128, 1024], F32, tag="ps")
                for hl in range(2):
                    p0 = hl * 64
                    nc.tensor.matmul(
                        ps[:, hl * 512:(hl + 1) * 512],
                        qk[p0:p0 + 64, pr * S + i * 128:pr * S + (i + 1) * 128],
                        qk[p0:p0 + 64,
                           (2 + pr) * S + sqh * SQC:(2 + pr) * S + (sqh + 1) * SQC],
                        start=True, stop=True)
                ex = exp_pool.tile([128, 1024], BF16, tag="ex", name=f"ex{i}")
                nc.scalar.activation(ex[:], ps[:], AF.Exp, scale=SCALE_INV)
                return ex

            def tpose_qb(qbg):
                """Transpose one q-block's [q, hd] pair; single strided evac."""
                tp = tp_pool.tile([128, 256], BF16, tag="tp")
                for ch in range(2):
                    nc.tensor.transpose(
                        tp[:, ch * 128:(ch + 1) * 128],
                        atn[:, qbg * 256 + ch * 128:qbg * 256 + (ch + 1) * 128],
                        ident[:])
                src = tp[:].rearrange("p (ch e) -> p ch e", ch=2)
                dst = atT[:].rearrange("p (ch g e) -> p ch g e", ch=2, e=128)[
                    :, :, qbg, :]
                nc.vector.tensor_copy(dst, src)

            def drain_group_thunks(sqh, pr, po, exs, tail=False):
                """Drain thunks: 8 PV accumulation groups (start->stop
                contiguous per bank; consecutive q-blocks alternate banks so
                the normalize read never stalls the next group), each q-block
                normalized as its pair completes; pr==1 appends the
                transposes and out-projections as separate pipeline phases."""
                thunks = []
                for qb in range(QB):
                    for hl in range(2):
                        def grp(qb=qb, hl=hl):
                            h = 2 * pr + hl
                            for i in range(SB):
                                nc.tensor.matmul(
                                    po[qb % 2][:, (qb // 2) * 130 + hl * 65:
                                               (qb // 2) * 130 + (hl + 1) * 65],
                                    exs[i][:, hl * SQC + qb * 128:
                                           hl * SQC + (qb + 1) * 128],
                                    vv[:, i * 260 + h * 65:
                                       i * 260 + (h + 1) * 65],
                                    start=(i == 0), stop=(i == SB - 1))
                            if hl == 1:
                                attn_norm_qb(sqh, pr, po, qb)
                                if pr == 1 and not tail:
                                    tpose_qb(sqh * QB + qb)
                                    outproj(sqh * QB + qb, nc.vector)
                        thunks.append(grp)
                if pr == 1 and tail:
                    # norms complete while later PV groups run; the
                    # transpose/out-proj chains then pipeline with the ps
                    # slots free for rotation
                    def chains():
                        for qb in range(QB):
                            tpose_qb(sqh * QB + qb)
                        for qb in range(QB):
                            outproj(sqh * QB + qb, nc.vector, tail=True)
                    thunks.append(chains)
                return thunks

            def attn_norm_qb(sqh, pr, po, qb):
                qbg = sqh * QB + qb
                pov = po[qb % 2][:, (qb // 2) * 130:(qb // 2 + 1) * 130] \
                    .rearrange("p (hl e) -> p hl e", hl=2)
                rc = rc_pool.tile([128, 2], F32, tag="rc")
                rcv = rc[:].rearrange("p (a b) -> p a b", a=2)
                nc.vector.reciprocal(rcv, pov[:, :, 64:65])
                for hl in range(2):
                    h = 2 * pr + hl
                    nc.vector.tensor_scalar_mul(
                        atn[:, qbg * 256 + h * 64:qbg * 256 + (h + 1) * 64],
                        pov[:, hl, 0:64], rc[:, hl:hl + 1])

            # ---- emission: software-pipelined projection+attention ----
            # PE executes its stream in order, so everything is emitted as
            # micro-hooks inside the scores/exp streams: projection chunks
            # (deadline-ordered) in passes 0-2, PV-drain groups of pass n-2
            # in passes 2-6, drains of passes 5 AND 6 sharing pass 7's
            # stream (sequentially, to keep PSUM accumulation groups of one
            # bank non-interleaved), and only pass 7's drain trailing.
            proj_hooks = {
                (0, 0): [(proj_qk, 1, 0)],     # k01 c1  (scores(0,4))
                (0, 3): [(proj_qk, 2, 0)],     # k01 c2  (scores(0,8); xt2 DMA)
                (0, 5): [(proj_qk, 0, 1)],     # k23 c0  (scores(1,0))
                (0, 7): [(proj_qk, 0, 3)],     # q23 c0  (scores(1,0))
                (0, 9): [(proj_qk, 3, 0)],     # k01 c3  (scores(0,12); xt3 DMA)
                (0, 11): [(proj_qk, 1, 1)],    # k23 c1  (scores(1,4))
                (0, 13): [(proj_qk, 2, 1)],    # k23 c2  (scores(1,8))
                (0, 15): [(proj_qk, 3, 1)],    # k23 c3  (scores(1,12))
                (1, 0): [(proj_v, 0, 0)],      # V chunks (PV(0), in pass 2)
                (1, 2): [(proj_v, 0, 1)],
                (1, 4): [(proj_v, 1, 0)],
                (1, 6): [(proj_v, 1, 1)],
                (1, 8): [(proj_v, 2, 0)],
                (1, 10): [(proj_v, 2, 1)],
                (1, 13): [(proj_qk, 1, 2)],    # q01 c1  (scores(2,0))
                (2, 0): [(proj_v, 3, 0)],
                (2, 2): [(proj_v, 3, 1)],
                (2, 13): [(proj_qk, 1, 3)],    # q23 c1  (scores(3,0))
                (3, 13): [(proj_qk, 2, 2)],    # q01 c2  (scores(4,0))
                (4, 13): [(proj_qk, 2, 3)],    # q23 c2  (scores(5,0))
                (5, 13): [(proj_qk, 3, 2)],    # q01 c3  (scores(6,0))
                (6, 13): [(proj_qk, 3, 3)],    # q23 c3  (scores(7,0))
            }

            def claim_po():
                return [po_pool.tile([128, 260], F32, tag=f"po{j}",
                                     name=f"po{j}")
                        for j in range(2)]

            # PE p-state warmup: ~40 cheap transposes bridge the DMA load
            # latency so the real matmuls start at full clock.
            warm = pers.tile([128, 128], BF16, tag="warm")
            nc.vector.memset(warm[:], 1.0)
            for _ in range(20):
                tpw = tp_pool.tile([128, 256], BF16, tag="tp")
                nc.tensor.transpose(tpw[:, 0:128], warm[:], warm[:])

            proj_qk(0, 0, pool_tag="ps")    # k01 chunk 0 (idle ps slot)
            proj_qk(0, 2, pool_tag="ps")    # q01 chunk 0

            passes = []                     # exs per pass
            meta = []                       # (sqh, pr) per pass
            for idx in range(2 * SQH):
                sqh, pr = divmod(idx, 2)
                extra = {}
                for (pidx, i), fns in proj_hooks.items():
                    if pidx == idx:
                        for fn, a, b in fns:
                            extra.setdefault(i, []).append(
                                lambda fn=fn, a=a, b=b: fn(a, b))
                if 2 <= idx <= 6:
                    j = idx - 2
                    po_j = claim_po()
                    # pass 2 carries the V chunk-3 hooks at i=0/2; its drain
                    # groups must come after them (they read all of V')
                    off = 4 if idx == 2 else 1
                    step = 1 if idx == 2 else 2
                    for g, th in enumerate(
                            drain_group_thunks(*meta[j], po_j, passes[j])):
                        extra.setdefault(min(off + step * g, SB - 1), []).append(th)
                if idx == 7:
                    po_5 = claim_po()
                    for g, th in enumerate(
                            drain_group_thunks(*meta[5], po_5, passes[5])):
                        extra.setdefault(g // 2, []).append(th)
                    po_6 = claim_po()
                    for g, th in enumerate(
                            drain_group_thunks(*meta[6], po_6, passes[6])):
                        extra.setdefault(min(6 + g, SB - 1), []).append(th)
                exs = []
                for i in range(SB):
                    exs.append(attn_i_step(sqh, pr, i))
                    for th in extra.get(i, ()):
                        th()
                passes.append(exs)
                meta.append((sqh, pr))
            po_7 = claim_po()
            for th in drain_group_thunks(*meta[7], po_7, passes[7], tail=True):
                th()

            if taps:
                nc.sync.dma_start(tap_qk_d.ap(), qk[:])
                nc.sync.dma_start(tap_vv_d.ap(), vv[:])
                nc.sync.dma_start(tap_atn_d.ap(), atn[:])
                nc.sync.dma_start(tap_atT_d.ap(), atT[:])

    nc.compile()
    return nc


def shard_inputs(x, w_qkv, b_qkv, w_out, b_out):
    """Host-side prep: slice per core, cast to bf16, pre-transpose layouts."""
    x = np.asarray(x, dtype=np.float32)
    w_qkv = np.asarray(w_qkv, dtype=np.float32)
    b_qkv = np.asarray(b_qkv, dtype=np.float32)
    w_out = np.asarray(w_out, dtype=np.float32)
    b_out = np.asarray(b_out, dtype=np.float32)
    eye = np.eye(128, dtype=BF)

    def qkblk(W):  # [256, 1024] -> [p, half, c, m]
        return W.astype(BF).reshape(2, 128, KC, 128).transpose(3, 0, 2, 1)

    in_maps = []
    for core in range(N_CORES):
        b, hg = divmod(core, 4)
        sl = slice(hg * 256, (hg + 1) * 256)
        Wq = w_qkv[0 * DIN:1 * DIN][sl]
        Wk = w_qkv[1 * DIN:2 * DIN][sl]
        Wv = w_qkv[2 * DIN:3 * DIN][sl]
        bq_s = b_qkv[0 * DIN:1 * DIN][sl]
        bv_s = b_qkv[2 * DIN:3 * DIN][sl]
        Wo = w_out[:, sl]
        xt = np.ascontiguousarray(
            x[b].astype(BF).reshape(SC, CS, KC, 128)
            .transpose(3, 0, 2, 1).reshape(128, SC * KC * CS))
        wqk = np.ascontiguousarray(
            np.concatenate([qkblk(Wk), qkblk(Wq)], axis=1)
            .reshape(128, 4 * KC * 128))
        wv_h = np.ascontiguousarray(
            Wv.astype(BF).reshape(256, KC, 128).transpose(2, 1, 0)
            .reshape(128, KC * 256))
        wo_h = np.ascontiguousarray(
            Wo.astype(BF).reshape(DIN, 2, 128).transpose(2, 1, 0)
            .reshape(128, 2 * DIN))
        bq_h = np.ascontiguousarray(bq_s.reshape(2, 128).T, dtype=np.float32)
        bv_h = np.ascontiguousarray(
            np.tile(bv_s.astype(BF).reshape(1, 256), (128, 2)))
        in_maps.append({
            "xt": xt, "wqk": wqk, "wv": wv_h, "wo": wo_h,
            "bq": bq_h, "bv": bv_h, "ident": eye,
        })
    return in_maps


def gather_output(results, b_out):
    """Sum the 4 head-group partials per batch element (the TP all-reduce,
    done host-side) and apply the out-projection bias."""
    bo = np.asarray(b_out, dtype=np.float32).reshape(1, DIN)
    outs = []
    for b in range(2):
        acc = np.zeros((S, DIN), dtype=np.float32)
        for core in range(4 * b, 4 * b + 4):
            acc += results[core]["out"].astype(np.float32)
        outs.append(acc + bo)
    return np.stack(outs, axis=0)


_NC_CACHE = {}


def _get_nc():
    if "nc" not in _NC_CACHE:
        _NC_CACHE["nc"] = build_nc()
    return _NC_CACHE["nc"]


def kernel(x, w_qkv, b_qkv, w_out, b_out):
    nc = _get_nc()
    in_maps = shard_inputs(x, w_qkv, b_qkv, w_out, b_out)
    res = run_bass_kernel_spmd(nc, in_maps, core_ids=list(range(N_CORES)))
    return gather_output(res.results, b_out)


if __name__ == "__main__":
    rng = np.random.default_rng(0)
    x = rng.standard_normal((2, S, DIN), dtype=np.float32)
    w_qkv = rng.standard_normal((3 * DIN, DIN), dtype=np.float32) / 32.0
    b_qkv = rng.standard_normal(3 * DIN, dtype=np.float32) * 0.02
    w_out = rng.standard_normal((DIN, DIN), dtype=np.float32) / 32.0
    b_out = rng.standard_normal(DIN, dtype=np.float32) * 0.02
    out = kernel(x=x, w_qkv=w_qkv, b_qkv=b_qkv, w_out=w_out, b_out=b_out)
    print("out", out.shape, out.dtype, float(np.abs(out).mean()))
